# revision 67
# baseline (speedup 1.0000x reference)
"""Trainium2 Bass kernel for batched cross-attention (nn_Attention).

Problem (hardcoded shapes):
  x_inner [8, 256, 2048], x_outer [8, 256, 2048]  (B, C, L)
  Wq/Wk/Wv [128, 256], bq/bk/bv [128]             (D, C)
  q = einsum('bcl,dc->bld', x_inner, Wq) + bq
  k = einsum('bcl,dc->bld', x_outer, Wk) + bk
  v = einsum('bcl,dc->bld', x_outer, Wv) + bv
  out = softmax(q @ k^T / sqrt(D), axis=-1) @ v   -> [8, 2048, 128]

Sharding: pure data-parallel over batch, one batch element per NeuronCore
(8 cores). No collectives.

Per-core design.  A 2-pass (Lq halves) streaming attention paced by two
~equal streams: ScalarE's exp (32 x [128,1024] ~= 33us, the only engine
with exp) and the PE (~35us of bf16 matmul).  Everything else rides
their slack:

  - Math: softmax(q+bq, k+bk) == softmax over k of (q+bq)@k — bk adds a
    per-row constant that cancels, so K is projected WITHOUT bias.  bv
    rides into V (attention weights sum to 1 via our own denominator).
  - All bf16 (fp8 was tested and rejected: softmax rows that concentrate
    on a few keys don't average out fp8 noise; P or V in fp8e4m3 alone
    costs ~1.6-1.9e-2 max-rel error vs the 2e-2 gate).
  - V is projected directly in [Lk, D] layout (x-chunk stationary,
    Wv moving, bv broadcast-added on evacuation) — no PE transposes.
  - Denominator: P-tile pair sums on GpSimd (otherwise idle), a small
    add-tree on VectorE, so only 3 ones-stationary matmuls per pass
    (tree total, p14, p15); p14/p15 skip the tree so the critical tail
    exp(15) -> dn -> recip -> mul -> DMA is short; the last exp is
    itself split in half so normalization starts before av finishes.
  - PSUM (8 banks): score ring tag "s" 2x[128,1024]f32, one shared
    transient slot tag "x" (projections / denominators, serialized by
    the pool ring), and the AV accumulator tag "av".
  - Head: K h0 / Q p0 projections evacuate in halves on separate queues
    (K on GpSimd, Q on VectorE) so score(0) starts ASAP; K h0 borrows
    the "av" PSUM ring so its matmuls chain after the PE warm-up
    (stops the scheduler hoisting them into a DMA-wait).  A dummy
    SBUF-input exp pulls the ~1.3us ACT_TABLE_LOAD off the stream.
  - The host casts x/W to bf16 on the way in, pre-broadcasts bv, and
    transposes/upcasts out^T -> [L, D] f32 on the way out (pure
    layout/precision prep, like the batch scatter/gather).
Softmax max-subtraction is skipped: scores/sqrt(D) are ~N(0,1), so exp()
cannot overflow in fp32.
"""

import numpy as np

B, C, L, D = 8, 256, 2048, 128
F = 512          # half-pass width (one PSUM bank of fp32)
W2 = 2 * F       # 1024: Lq chunk per pass / exp tile width
LT = L // 128    # 16 Lk tiles
CK = C // 128    # 2 contraction chunks
SCALE = 1.0 / float(np.sqrt(D))

_COMPILED = None


def _build():
    import concourse.bass as bass
    import concourse.mybir as mybir
    import concourse.tile as tile
    from concourse import bacc
    from contextlib import ExitStack

    F32 = mybir.dt.float32
    BF16 = mybir.dt.bfloat16
    AFT = mybir.ActivationFunctionType
    ts = bass.ts

    nc = bacc.Bacc("TRN2", target_bir_lowering=False, debug=False, num_devices=8)

    xi_ext = nc.declare_dram_parameter("x_inner", [C, L], BF16, isOutput=False)
    xo_ext = nc.declare_dram_parameter("x_outer", [C, L], BF16, isOutput=False)
    # host pre-arranges W as [p=128, 3, CK, D]: the upload is one fully
    # contiguous DMA (the gather layout cost ~4us/matrix in 256B descriptors)
    w_ext = nc.declare_dram_parameter("W_all", [128, 3 * CK * D], BF16,
                                      isOutput=False)
    b_ext = nc.declare_dram_parameter("b_all", [D, 3], F32, isOutput=False)
    bvb_ext = nc.declare_dram_parameter("bv_bcast", [128, 8 * 128], BF16,
                                        isOutput=False)
    out_ext = nc.declare_dram_parameter("out", [D, L], BF16, isOutput=True)

    with tile.TileContext(nc) as tc:
        with ExitStack() as ctx:
            const = ctx.enter_context(tc.tile_pool(name="const", bufs=1))
            xin = ctx.enter_context(tc.tile_pool(name="xin", bufs=1))
            qkv = ctx.enter_context(tc.tile_pool(name="qkv", bufs=1))
            pts = ctx.enter_context(tc.tile_pool(name="pts", bufs=1))
            work = ctx.enter_context(tc.tile_pool(name="work", bufs=2))
            ps = ctx.enter_context(tc.tile_pool(name="ps", bufs=1, space="PSUM"))

            # ---- constants --------------------------------------------------
            w_all = const.tile([128, 3, CK, D], BF16, tag="w")
            b_all = const.tile([D, 3], F32, tag="b")    # DMA deferred below
            bvb = const.tile([128, 8 * 128], BF16, tag="bvb")  # DMA deferred
            ones_bf = const.tile([128, 128], BF16, tag="ones_bf")
            warm_sb = const.tile([D, 3], BF16, tag="warm_sb")

            # ---- X loads.  HW aggregate input bandwidth is ~130-150 GB/s,
            # so arrival ORDER dominates the head: the h0 halves of x are
            # split into [128,512] quarters and issued strictly in need
            # order (Q/K first quarters -> first exp after ~512KB landed).
            # h1 halves stay as [128,1024] tiles issued after.
            xq = {("xi", 0): [], ("xi", 1): [], ("xo", 0): [], ("xo", 1): []}
            for q in range(2):
                for nm, ext in (("xi", xi_ext), ("xo", xo_ext)):
                    for c, eng in ((0, nc.sync), (1, nc.gpsimd)):
                        t = xin.tile([128, F], BF16, tag=f"q{nm}{c}{q}",
                                     name=f"q{nm}{c}{q}")
                        eng.dma_start(
                            out=t[:],
                            in_=ext[c * 128:(c + 1) * 128, ts(q, F)],
                        )
                        xq[(nm, c)].append(t)
            # small consts + h1 halves after the critical quarters
            nc.sync.dma_start(out=b_all[:], in_=b_ext[:])
            nc.sync.dma_start(
                out=w_all[:].rearrange("p a b c -> p (a b c)"),
                in_=w_ext[:],
            )
            nc.gpsimd.memset(ones_bf[:], 1.0)
            xo_t = [[None] * 2 for _ in range(CK)]
            xi_t = [[None] * 2 for _ in range(CK)]
            qmap = {("xo", 0): nc.sync, ("xo", 1): nc.gpsimd,
                    ("xi", 0): nc.scalar, ("xi", 1): nc.scalar}
            for nm, tiles, ext in (("xo", xo_t, xo_ext), ("xi", xi_t, xi_ext)):
                for c in range(CK):
                    t = xin.tile([128, W2], BF16, tag=f"{nm}{c}1",
                                 name=f"{nm}{c}1")
                    qmap[(nm, c)].dma_start(
                        out=t[:],
                        in_=ext[c * 128:(c + 1) * 128, ts(1, L // 2)],
                    )
                    tiles[c][1] = t
            nc.sync.dma_start(out=bvb[:], in_=bvb_ext[:])
            # dummy exp (input: the just-landed biases) pulls the ~1.3us
            # ACT_TABLE_LOAD off the critical exp stream
            nc.scalar.activation(warm_sb[:], b_all[:], AFT.Exp, scale=1.0)

            # ---- persistent SBUF operands ----------------------------------
            kt = qkv.tile([128, L], BF16, tag="kt")      # K^T (no bias)
            qt = qkv.tile([128, L], BF16, tag="qt")      # Q^T (+bq)
            v_blk = [qkv.tile([128, 8 * 128], BF16, tag=f"vb{g}", name=f"vb{g}")
                     for g in range(2)]                  # V tiles [Lk, D] (+bv)

            def proj(w_idx, xs, h, out_ap, b_idx, ptag, eng, split=False):
                """[128,1024] projection in [d, l] layout: PSUM accum over C,
                evacuation on `eng` (bias fused if b_idx).  split=True
                evacuates each half as soon as its matmuls are done."""
                p = ps.tile([128, W2], F32, tag=ptag, name=f"proj{w_idx}{h}",
                            bufs=(2 if ptag == "s" else 1))

                def evac(ap_out, ap_in):
                    if b_idx is None:
                        eng.tensor_copy(ap_out, ap_in)
                    else:
                        eng.tensor_scalar_add(ap_out, ap_in,
                                              b_all[:, b_idx:b_idx + 1])

                for hh in range(2):
                    for c in range(CK):
                        nc.tensor.matmul(
                            p[:, ts(hh, F)],
                            w_all[:, w_idx, c, :],
                            xs[c][h][:, ts(hh, F)],
                            start=(c == 0), stop=(c == CK - 1),
                        )
                    if split:
                        evac(out_ap[:, ts(hh, F)], p[:, ts(hh, F)])
                if not split:
                    evac(out_ap, p[:])

            def v_group(g, ptag):
                """8 V tiles [128l, 128d] projected directly: x-chunk
                stationary, Wv moving; bv broadcast-added on evacuation."""
                p = ps.tile([128, 8 * 128], F32, tag=ptag, name=f"vps{g}")
                for j in range(8):
                    for c in range(CK):
                        if g == 0:
                            sta = xq[("xo", c)][j // 4][:, ts(j % 4, 128)]
                        else:
                            sta = xo_t[c][1][:, ts(j, 128)]
                        nc.tensor.matmul(
                            p[:, ts(j, 128)],
                            sta,
                            w_all[:, 2, c, :],
                            start=(c == 0), stop=(c == CK - 1),
                        )
                nc.vector.tensor_add(v_blk[g][:], p[:], bvb[:])

            # ---- attention machinery ---------------------------------------
            p_tiles = [[None] * LT for _ in range(2)]
            pair_s = [[None] * 7 for _ in range(2)]
            quad_s = [[None] * 3 for _ in range(2)]
            tsum_s = [[None] * 3 for _ in range(2)]
            av_t = [None, None]
            dn_t = [None, None]

            def score(pr, t):
                sp = ps.tile([128, W2], F32, tag="s", name="s_ps", bufs=2)
                for hh in range(2):
                    nc.tensor.matmul(
                        sp[:, ts(hh, F)],
                        kt[:, ts(t, 128)],
                        qt[:, pr * W2 + hh * F: pr * W2 + (hh + 1) * F],
                        start=True, stop=True,
                    )
                pt = pts.tile([128, W2], BF16, tag=f"p{pr}_{t}", name=f"p{pr}_{t}")
                p_tiles[pr][t] = pt
                return sp, pt

            def score_exp(pr, t):
                sp, pt = score(pr, t)
                nc.scalar.activation(pt[:], sp[:], AFT.Exp, scale=SCALE)

            def score_exp_split(pr, t):
                # last tile of a pass: exp in halves so av/dn/normalize
                # start after half the tile
                sp, pt = score(pr, t)
                for hh in range(2):
                    nc.scalar.activation(pt[:, ts(hh, F)], sp[:, ts(hh, F)],
                                         AFT.Exp, scale=SCALE)

            def av_mm(pr, t, only_h=None):
                if t == 0:
                    av_t[pr] = ps.tile([128, W2], F32, tag="av", name=f"av{pr}")
                for hh in range(2) if only_h is None else (only_h,):
                    nc.tensor.matmul(
                        av_t[pr][:, ts(hh, F)],
                        v_blk[t // 8][:, ts(t % 8, 128)],
                        p_tiles[pr][t][:, ts(hh, F)],
                        start=(t == 0), stop=(t == LT - 1),
                    )

            def add_t(dst_list, i, a, b, eng, nm):
                sm = pts.tile([128, W2], BF16, tag=f"{nm}{i}", name=f"{nm}{i}")
                eng.tensor_add(sm[:], a[:], b[:])
                dst_list[i] = sm

            def dn_mm(pr, src, start, stop, only_h=None):
                if start:
                    dn_t[pr] = ps.tile([128, W2], F32, tag="x", name=f"dn{pr}")
                for hh in range(2) if only_h is None else (only_h,):
                    nc.tensor.matmul(
                        dn_t[pr][:, ts(hh, F)],
                        ones_bf[:],
                        src[:, ts(hh, F)],
                        start=start, stop=stop,
                    )

            def normalize_h(pr, hh, mul_eng=None):
                recip = work.tile([128, F], F32, tag="recip", name="recip")
                nc.vector.reciprocal_approx_fast(recip[:], dn_t[pr][:, ts(hh, F)])
                o = work.tile([128, F], BF16, tag="o", name="o")
                (mul_eng or nc.vector).tensor_mul(o[:], av_t[pr][:, ts(hh, F)],
                                                  recip[:])
                nc.sync.dma_start(out=out_ext[:, ts(2 * pr + hh, F)], in_=o[:])

            def tree(pr, t):
                """denominator add-tree over tiles 0..11: pairs on GpSimd,
                quads on DVE; quads + tiles 12..15 feed the denominator
                matmuls, spread one per period across the pass."""
                pr_ = pair_s[pr]
                if t >= 2 and t % 2 == 0 and t // 2 - 1 < 6:
                    m = t // 2 - 1
                    # HW GpSimd adds cost ~2.3us each: split pairs between
                    # GpSimd (0-2) and DVE (3-5) so neither serializes
                    add_t(pr_, m, p_tiles[pr][2 * m], p_tiles[pr][2 * m + 1],
                          nc.gpsimd if m < 3 else nc.vector, f"pr{pr}_")
                if t == 5:
                    add_t(quad_s[pr], 0, pr_[0], pr_[1], nc.vector, f"qd{pr}_")
                elif t == 9:
                    add_t(quad_s[pr], 1, pr_[2], pr_[3], nc.vector, f"qd{pr}_")
                elif t == 11:
                    add_t(tsum_s[pr], 0, quad_s[pr][0], quad_s[pr][1],
                          nc.vector, f"ts{pr}_")
                elif t == 13:
                    add_t(quad_s[pr], 2, pr_[4], pr_[5], nc.vector, f"qd{pr}_")
                elif t == 14:
                    add_t(tsum_s[pr], 1, tsum_s[pr][0], quad_s[pr][2],
                          nc.vector, f"ts{pr}_")
                    dn_mm(pr, p_tiles[pr][12][:], True, False)
                elif t == 15:
                    dn_mm(pr, p_tiles[pr][13][:], False, False)

            # ---- static schedule -------------------------------------------
            # Up-front: K h0 (GpSimd evac) and Q p0 (DVE evac) feed
            # score(0) ASAP.  Each half gets its OWN PSUM tile (PSUM deps
            # are tile-granular: a shared tile would serialize the h1
            # matmuls behind the h0 evacuation).  K h0's first half rides
            # the "av" ring so its matmuls chain after the PE warm-up
            # (stops the scheduler hoisting them into a DMA-wait).
            def q_proj(w_idx, nm, q, out_sb, b_idx, tag, bufs_, eng):
                """one [128,512] quarter-projection from quarter x tiles"""
                p = ps.tile([128, F], F32, tag=tag, name=f"hp{w_idx}{q}",
                            bufs=bufs_)
                for c in range(CK):
                    nc.tensor.matmul(
                        p[:], w_all[:, w_idx, c, :], xq[(nm, c)][q][:],
                        start=(c == 0), stop=(c == CK - 1),
                    )
                if b_idx is None:
                    # ScalarE Copy: the ACT queue is idle pre-stream
                    eng.copy(out_sb, p[:])
                else:
                    eng.tensor_scalar_add(out_sb, p[:], b_all[:, b_idx:b_idx + 1])

            # emission in need order: K q0 + Q q0 gate exp0-h0
            q_proj(1, "xo", 0, kt[:, 0:F], None, "av", 1, nc.scalar)   # K q0
            q_proj(0, "xi", 0, qt[:, 0:F], 0, "s", 2, nc.vector)       # Q q0
            q_proj(1, "xo", 1, kt[:, F:W2], None, "x", 1, nc.scalar)   # K q1
            q_proj(0, "xi", 1, qt[:, F:W2], 0, "s", 2, nc.vector)      # Q q1

            def half_proj(w_idx, xs, hh, out_sb, b_idx, eng):
                """one [128,512] half of an h1 projection in the "x" slot"""
                p = ps.tile([128, F], F32, tag="x", name=f"xp{w_idx}{hh}")
                for c in range(CK):
                    nc.tensor.matmul(
                        p[:], w_all[:, w_idx, c, :], xs[c][1][:, ts(hh, F)],
                        start=(c == 0), stop=(c == CK - 1),
                    )
                dst = out_sb[:, W2 + hh * F: W2 + (hh + 1) * F]
                if b_idx is None:
                    eng.tensor_copy(dst, p[:])
                else:
                    eng.tensor_scalar_add(dst, p[:], b_all[:, b_idx:b_idx + 1])

            # ---- pass 0 ----
            # score(0) uses two separate PSUM half-tiles: PSUM deps are
            # tile-granular, so exp0-h0 starts without waiting the h1 matmul
            pt0 = pts.tile([128, W2], BF16, tag="p0_0", name="p0_0")
            p_tiles[0][0] = pt0
            for hh in range(2):
                sp_h = ps.tile([128, F], F32, tag="s", name=f"s0h{hh}", bufs=2)
                nc.tensor.matmul(
                    sp_h[:], kt[:, 0:128], qt[:, hh * F:(hh + 1) * F],
                    start=True, stop=True,
                )
                nc.scalar.activation(pt0[:, ts(hh, F)], sp_h[:], AFT.Exp,
                                     scale=SCALE)
            for t in range(1, LT):
                if t == LT - 1:
                    # split exp at t=15 shortens the tail
                    score_exp_split(0, t)
                else:
                    score_exp(0, t)
                # AV: starts once v_blk[0] is ready
                if t == 3:
                    for m in range(3):
                        av_mm(0, m)
                elif t >= 4:
                    av_mm(0, t - 1)
                tree(0, t)
                # pass-1 projections ride the PE slack in the "x" slot
                if t == 1:
                    v_group(0, "av")   # av-ring: chains after K h0's first half
                elif t == 3:
                    proj(1, xo_t, 1, kt[:, W2:L], None, "x", nc.vector)  # K h1
                elif t == 5:
                    v_group(1, "x")
                elif t == 9:
                    proj(0, xi_t, 1, qt[:, W2:L], 0, "x", nc.vector)     # Q p1

            # ---- boundary: keep ScalarE fed while pass 0 drains ----
            score_exp(1, 0)
            dn_mm(0, tsum_s[0][1][:], False, False)
            dn_mm(0, p_tiles[0][14][:], False, False)
            score_exp(1, 1)
            av_mm(0, LT - 1, only_h=0)
            dn_mm(0, p_tiles[0][15][:], False, True, only_h=0)
            score_exp(1, 2)
            av_mm(0, LT - 1, only_h=1)
            dn_mm(0, p_tiles[0][15][:], False, True, only_h=1)
            normalize_h(0, 0)
            normalize_h(0, 1)
            tree(1, 2)

            # ---- pass 1 ----
            AV1 = {5: (0,), 6: (1, 2), 7: (3, 4), 8: (5, 6), 9: (7, 8)}
            for t in range(3, LT):
                if t == LT - 1:
                    score_exp_split(1, t)
                else:
                    score_exp(1, t)
                for m in AV1.get(t, (t - 1,) if t >= 10 else ()):
                    av_mm(1, m)
                tree(1, t)
            # tail: only p15's av/dn trail the last exp; muls ride GpSimd
            # so the two reciprocals are back-to-back on DVE
            dn_mm(1, tsum_s[1][1][:], False, False)
            dn_mm(1, p_tiles[1][14][:], False, False)
            av_mm(1, LT - 1, only_h=0)
            dn_mm(1, p_tiles[1][15][:], False, True, only_h=0)
            av_mm(1, LT - 1, only_h=1)
            dn_mm(1, p_tiles[1][15][:], False, True, only_h=1)
            normalize_h(1, 0)
            normalize_h(1, 1)

    nc.compile()
    return nc


def _in_maps(inputs):
    import ml_dtypes

    bf16 = ml_dtypes.bfloat16
    x_inner = np.ascontiguousarray(np.asarray(inputs["x_inner"]).astype(bf16))
    x_outer = np.ascontiguousarray(np.asarray(inputs["x_outer"]).astype(bf16))
    # device layout [p=128, 3, CK, D]: W^T[c, d] split as c = j*128 + p
    w_t = np.stack([
        np.asarray(inputs["Wq"]).astype(np.float32).T,
        np.asarray(inputs["Wk"]).astype(np.float32).T,
        np.asarray(inputs["Wv"]).astype(np.float32).T,
    ])  # [3, C, D]
    w_all = np.ascontiguousarray(
        w_t.reshape(3, C // 128, 128, D).transpose(2, 0, 1, 3)
        .reshape(128, -1).astype(bf16))
    b_all = np.ascontiguousarray(np.stack([
        np.asarray(inputs["bq"], dtype=np.float32),
        np.asarray(inputs["bk"], dtype=np.float32),
        np.asarray(inputs["bv"], dtype=np.float32),
    ], axis=1))
    bv_bcast = np.ascontiguousarray(
        np.tile(np.asarray(inputs["bv"], dtype=np.float32).astype(bf16),
                (128, 8)))
    return [
        {
            "x_inner": x_inner[b],
            "x_outer": x_outer[b],
            "W_all": w_all,
            "b_all": b_all,
            "bv_bcast": bv_bcast,
        }
        for b in range(B)
    ]


def kernel(**inputs):
    global _COMPILED
    from concourse.bass_utils import run_bass_kernel_spmd

    if _COMPILED is None:
        _COMPILED = _build()
    in_maps = _in_maps(inputs)
    res = run_bass_kernel_spmd(_COMPILED, in_maps, core_ids=list(range(B)))
    # device emits bf16 out^T [D, L]; transpose/upcast on host (pure layout)
    return np.stack(
        [res.results[b]["out"].T.astype(np.float32) for b in range(B)]
    )


# revision 72
# speedup vs baseline: 1.0361x; 1.0361x over previous
"""Trainium2 Bass kernel for batched cross-attention (nn_Attention).

Problem (hardcoded shapes):
  x_inner [8, 256, 2048], x_outer [8, 256, 2048]  (B, C, L)
  Wq/Wk/Wv [128, 256], bq/bk/bv [128]             (D, C)
  q = einsum('bcl,dc->bld', x_inner, Wq) + bq
  k = einsum('bcl,dc->bld', x_outer, Wk) + bk
  v = einsum('bcl,dc->bld', x_outer, Wv) + bv
  out = softmax(q @ k^T / sqrt(D), axis=-1) @ v   -> [8, 2048, 128]

Sharding: pure data-parallel over batch, one batch element per NeuronCore
(8 cores). No collectives.

Per-core design.  A 2-pass (Lq halves) streaming attention paced by two
~equal streams: ScalarE's exp (32 x [128,1024] ~= 33us, the only engine
with exp) and the PE (~35us of bf16 matmul).  Everything else rides
their slack:

  - Math: softmax(q+bq, k+bk) == softmax over k of (q+bq)@k — bk adds a
    per-row constant that cancels, so K is projected WITHOUT bias.  bv
    rides into V (attention weights sum to 1 via our own denominator).
  - All bf16 (fp8 was tested and rejected: softmax rows that concentrate
    on a few keys don't average out fp8 noise; P or V in fp8e4m3 alone
    costs ~1.6-1.9e-2 max-rel error vs the 2e-2 gate).
  - V is projected directly in [Lk, D] layout (x-chunk stationary,
    Wv moving, bv broadcast-added on evacuation) — no PE transposes.
  - Denominator: P-tile pair sums on GpSimd (otherwise idle), a small
    add-tree on VectorE, so only 3 ones-stationary matmuls per pass
    (tree total, p14, p15); p14/p15 skip the tree so the critical tail
    exp(15) -> dn -> recip -> mul -> DMA is short; the last exp is
    itself split in half so normalization starts before av finishes.
  - PSUM (8 banks): score ring tag "s" 2x[128,1024]f32, one shared
    transient slot tag "x" (projections / denominators, serialized by
    the pool ring), and the AV accumulator tag "av".
  - Head: K h0 / Q p0 projections evacuate in halves on separate queues
    (K on GpSimd, Q on VectorE) so score(0) starts ASAP; K h0 borrows
    the "av" PSUM ring so its matmuls chain after the PE warm-up
    (stops the scheduler hoisting them into a DMA-wait).  A dummy
    SBUF-input exp pulls the ~1.3us ACT_TABLE_LOAD off the stream.
  - The host casts x/W to bf16 on the way in, pre-broadcasts bv, and
    transposes/upcasts out^T -> [L, D] f32 on the way out (pure
    layout/precision prep, like the batch scatter/gather).
Softmax max-subtraction is skipped: scores/sqrt(D) are ~N(0,1), so exp()
cannot overflow in fp32.
"""

import numpy as np

B, C, L, D = 8, 256, 2048, 128
F = 512          # half-pass width (one PSUM bank of fp32)
W2 = 2 * F       # 1024: Lq chunk per pass / exp tile width
LT = L // 128    # 16 Lk tiles
CK = C // 128    # 2 contraction chunks
SCALE = 1.0 / float(np.sqrt(D))

_COMPILED = None


def _build():
    import concourse.bass as bass
    import concourse.mybir as mybir
    import concourse.tile as tile
    from concourse import bacc
    from contextlib import ExitStack

    F32 = mybir.dt.float32
    BF16 = mybir.dt.bfloat16
    AFT = mybir.ActivationFunctionType
    ts = bass.ts

    nc = bacc.Bacc("TRN2", target_bir_lowering=False, debug=False, num_devices=8)

    xi_ext = nc.declare_dram_parameter("x_inner", [C, L], BF16, isOutput=False)
    xo_ext = nc.declare_dram_parameter("x_outer", [C, L], BF16, isOutput=False)
    # host pre-arranges W as [p=128, 3, CK, D]: the upload is one fully
    # contiguous DMA (the gather layout cost ~4us/matrix in 256B descriptors)
    w_ext = nc.declare_dram_parameter("W_all", [128, 3 * CK * D], BF16,
                                      isOutput=False)
    b_ext = nc.declare_dram_parameter("b_all", [D, 3], F32, isOutput=False)
    out_ext = nc.declare_dram_parameter("out", [D, L], BF16, isOutput=True)

    with tile.TileContext(nc) as tc:
        with ExitStack() as ctx:
            const = ctx.enter_context(tc.tile_pool(name="const", bufs=1))
            xin = ctx.enter_context(tc.tile_pool(name="xin", bufs=1))
            qkv = ctx.enter_context(tc.tile_pool(name="qkv", bufs=1))
            pts = ctx.enter_context(tc.tile_pool(name="pts", bufs=1))
            work = ctx.enter_context(tc.tile_pool(name="work", bufs=2))
            ps = ctx.enter_context(tc.tile_pool(name="ps", bufs=1, space="PSUM"))

            # ---- constants --------------------------------------------------
            w_all = const.tile([128, 3, CK, D], BF16, tag="w")
            b_all = const.tile([D, 3], F32, tag="b")    # DMA deferred below
            ones_bf = const.tile([128, 128], BF16, tag="ones_bf")
            warm_sb = const.tile([D, 3], BF16, tag="warm_sb")
            warm_src = const.tile([128, F], BF16, tag="warm")
            nc.gpsimd.memset(ones_bf[:], 1.0)
            nc.gpsimd.memset(warm_src[:], 0.0)

            # ---- X loads.  HW aggregate input bandwidth is ~150-280 GB/s
            # and data only starts flowing ~8us in, so arrival ORDER
            # dominates the head: wk first, then the h0 halves of x as
            # [128,512] quarters in need order (Q/K first quarters ->
            # first exp after ~0.6MB landed).  h1 halves follow as
            # [128,1024] tiles.
            nc.sync.dma_start(out=w_all[:, 1].rearrange("p b c -> p (b c)"),
                              in_=w_ext[:, 256:512])         # wk
            xq = {("xi", 0): [], ("xi", 1): [], ("xo", 0): [], ("xo", 1): []}
            for q in range(2):
                for nm, ext in (("xi", xi_ext), ("xo", xo_ext)):
                    for c, eng in ((0, nc.sync), (1, nc.gpsimd)):
                        t = xin.tile([128, F], BF16, tag=f"q{nm}{c}{q}",
                                     name=f"q{nm}{c}{q}")
                        eng.dma_start(
                            out=t[:],
                            in_=ext[c * 128:(c + 1) * 128, ts(q, F)],
                        )
                        xq[(nm, c)].append(t)
                if q == 0:
                    nc.sync.dma_start(
                        out=w_all[:, 0].rearrange("p b c -> p (b c)"),
                        in_=w_ext[:, 0:256])                  # wq
                    nc.sync.dma_start(out=b_all[:], in_=b_ext[:])
            nc.sync.dma_start(out=w_all[:, 2].rearrange("p b c -> p (b c)"),
                              in_=w_ext[:, 512:768])          # wv
            xo_t = [[None] * 2 for _ in range(CK)]
            xi_t = [[None] * 2 for _ in range(CK)]
            qmap = {("xo", 0): nc.sync, ("xo", 1): nc.gpsimd,
                    ("xi", 0): nc.scalar, ("xi", 1): nc.scalar}
            for nm, tiles, ext in (("xo", xo_t, xo_ext), ("xi", xi_t, xi_ext)):
                for c in range(CK):
                    t = xin.tile([128, W2], BF16, tag=f"{nm}{c}1",
                                 name=f"{nm}{c}1")
                    qmap[(nm, c)].dma_start(
                        out=t[:],
                        in_=ext[c * 128:(c + 1) * 128, ts(1, L // 2)],
                    )
                    tiles[c][1] = t
            # dummy exp (input: the just-landed biases) pulls the ~1.3us
            # ACT_TABLE_LOAD off the critical exp stream
            nc.scalar.activation(warm_sb[:], b_all[:], AFT.Exp, scale=1.0)

            # ---- PE warm-up: the input DMAs take ~8us; keep the PE busy
            # the whole time so it ramps to 2.4GHz before the real work
            # (at the 1.2GHz mid-pstate the projection inserts overflow
            # the exp-period budget and the stream develops gaps).
            wp = ps.tile([128, W2], F32, tag="av", name="warm_ps")
            for _ in range(26):
                nc.tensor.matmul(wp[:, 0:F], ones_bf[:], warm_src[:],
                                 start=True, stop=True)

            # ---- persistent SBUF operands ----------------------------------
            kt = qkv.tile([128, L], BF16, tag="kt")      # K^T (no bias)
            qt = qkv.tile([128, L], BF16, tag="qt")      # Q^T (+bq)
            v_blk = [qkv.tile([128, 8 * 128], BF16, tag=f"vb{g}", name=f"vb{g}")
                     for g in range(2)]                  # V tiles [Lk, D] (+bv)

            def proj(w_idx, xs, h, out_ap, b_idx, ptag, eng, split=False):
                """[128,1024] projection in [d, l] layout: PSUM accum over C,
                evacuation on `eng` (bias fused if b_idx).  split=True
                evacuates each half as soon as its matmuls are done."""
                p = ps.tile([128, W2], F32, tag=ptag, name=f"proj{w_idx}{h}",
                            bufs=(2 if ptag == "s" else 1))

                def evac(ap_out, ap_in):
                    if b_idx is None:
                        eng.tensor_copy(ap_out, ap_in)
                    else:
                        eng.tensor_scalar_add(ap_out, ap_in,
                                              b_all[:, b_idx:b_idx + 1])

                for hh in range(2):
                    for c in range(CK):
                        nc.tensor.matmul(
                            p[:, ts(hh, F)],
                            w_all[:, w_idx, c, :],
                            xs[c][h][:, ts(hh, F)],
                            start=(c == 0), stop=(c == CK - 1),
                        )
                    if split:
                        evac(out_ap[:, ts(hh, F)], p[:, ts(hh, F)])
                if not split:
                    evac(out_ap, p[:])

            def v_group(g, ptag):
                """8 V tiles [128l, 128d] projected directly: x-chunk
                stationary, Wv moving; bv broadcast-added on evacuation."""
                p = ps.tile([128, 8 * 128], F32, tag=ptag, name=f"vps{g}")
                for j in range(8):
                    for c in range(CK):
                        if g == 0:
                            sta = xq[("xo", c)][j // 4][:, ts(j % 4, 128)]
                        else:
                            sta = xo_t[c][1][:, ts(j, 128)]
                        nc.tensor.matmul(
                            p[:, ts(j, 128)],
                            sta,
                            w_all[:, 2, c, :],
                            start=(c == 0), stop=(c == CK - 1),
                        )
                nc.vector.tensor_copy(v_blk[g][:], p[:])

            # ---- attention machinery ---------------------------------------
            p_tiles = [[None] * LT for _ in range(2)]
            pair_s = [[None] * 7 for _ in range(2)]
            quad_s = [[None] * 3 for _ in range(2)]
            tsum_s = [[None] * 3 for _ in range(2)]
            av_t = [None, None]
            dn_t = [None, None]

            def score(pr, t):
                sp = ps.tile([128, W2], F32, tag="s", name="s_ps", bufs=2)
                for hh in range(2):
                    nc.tensor.matmul(
                        sp[:, ts(hh, F)],
                        kt[:, ts(t, 128)],
                        qt[:, pr * W2 + hh * F: pr * W2 + (hh + 1) * F],
                        start=True, stop=True,
                    )
                pt = pts.tile([128, W2], BF16, tag=f"p{pr}_{t}", name=f"p{pr}_{t}")
                p_tiles[pr][t] = pt
                return sp, pt

            def score_exp(pr, t):
                sp, pt = score(pr, t)
                nc.scalar.activation(pt[:], sp[:], AFT.Exp, scale=SCALE)

            def score_exp_split(pr, t):
                # last tile of a pass: exp in halves so av/dn/normalize
                # start after half the tile
                sp, pt = score(pr, t)
                for hh in range(2):
                    nc.scalar.activation(pt[:, ts(hh, F)], sp[:, ts(hh, F)],
                                         AFT.Exp, scale=SCALE)

            def av_mm(pr, t, only_h=None):
                if t == 0:
                    av_t[pr] = ps.tile([128, W2], F32, tag="av", name=f"av{pr}")
                for hh in range(2) if only_h is None else (only_h,):
                    nc.tensor.matmul(
                        av_t[pr][:, ts(hh, F)],
                        v_blk[t // 8][:, ts(t % 8, 128)],
                        p_tiles[pr][t][:, ts(hh, F)],
                        start=(t == 0), stop=(t == LT - 1),
                    )

            def add_t(dst_list, i, a, b, eng, nm):
                sm = pts.tile([128, W2], BF16, tag=f"{nm}{i}", name=f"{nm}{i}")
                eng.tensor_add(sm[:], a[:], b[:])
                dst_list[i] = sm

            def dn_mm(pr, src, start, stop, only_h=None):
                if start:
                    dn_t[pr] = ps.tile([128, W2], F32, tag="x", name=f"dn{pr}")
                for hh in range(2) if only_h is None else (only_h,):
                    nc.tensor.matmul(
                        dn_t[pr][:, ts(hh, F)],
                        ones_bf[:],
                        src[:, ts(hh, F)],
                        start=start, stop=stop,
                    )

            def normalize_h(pr, hh):
                recip = work.tile([128, F], F32, tag="recip", name="recip")
                nc.vector.reciprocal_approx_fast(recip[:], dn_t[pr][:, ts(hh, F)])
                o = work.tile([128, F], BF16, tag="o", name="o")
                nc.vector.tensor_mul(o[:], av_t[pr][:, ts(hh, F)], recip[:])
                # out^T is [d, q]: bv is a per-partition scalar here, so the
                # V bias lands as one tensor_scalar add (no bv broadcast DMA)
                nc.vector.tensor_scalar_add(o[:], o[:], b_all[:, 2:3])
                nc.sync.dma_start(out=out_ext[:, ts(2 * pr + hh, F)], in_=o[:])

            def tree(pr, t):
                """denominator add-tree over tiles 0..11: pairs on GpSimd,
                quads on DVE; quads + tiles 12..15 feed the denominator
                matmuls, spread one per period across the pass."""
                pr_ = pair_s[pr]
                if t >= 2 and t % 2 == 0 and t // 2 - 1 < 6:
                    m = t // 2 - 1
                    # HW GpSimd adds cost ~2.3us each: split pairs between
                    # GpSimd (0-2) and DVE (3-5) so neither serializes
                    add_t(pr_, m, p_tiles[pr][2 * m], p_tiles[pr][2 * m + 1],
                          nc.gpsimd if m < 3 else nc.vector, f"pr{pr}_")
                if t == 5:
                    add_t(quad_s[pr], 0, pr_[0], pr_[1], nc.vector, f"qd{pr}_")
                elif t == 9:
                    add_t(quad_s[pr], 1, pr_[2], pr_[3], nc.vector, f"qd{pr}_")
                elif t == 11:
                    add_t(tsum_s[pr], 0, quad_s[pr][0], quad_s[pr][1],
                          nc.vector, f"ts{pr}_")
                elif t == 13:
                    add_t(quad_s[pr], 2, pr_[4], pr_[5], nc.vector, f"qd{pr}_")
                elif t == 14:
                    add_t(tsum_s[pr], 1, tsum_s[pr][0], quad_s[pr][2],
                          nc.vector, f"ts{pr}_")
                    dn_mm(pr, p_tiles[pr][12][:], True, False)
                elif t == 15:
                    dn_mm(pr, p_tiles[pr][13][:], False, False)

            # ---- static schedule -------------------------------------------
            # Up-front: K h0 (GpSimd evac) and Q p0 (DVE evac) feed
            # score(0) ASAP.  Each half gets its OWN PSUM tile (PSUM deps
            # are tile-granular: a shared tile would serialize the h1
            # matmuls behind the h0 evacuation).  K h0's first half rides
            # the "av" ring so its matmuls chain after the PE warm-up
            # (stops the scheduler hoisting them into a DMA-wait).
            def q_proj(w_idx, nm, q, out_sb, b_idx, tag, bufs_, eng):
                """one [128,512] quarter-projection from quarter x tiles"""
                p = ps.tile([128, F], F32, tag=tag, name=f"hp{w_idx}{q}",
                            bufs=bufs_)
                for c in range(CK):
                    nc.tensor.matmul(
                        p[:], w_all[:, w_idx, c, :], xq[(nm, c)][q][:],
                        start=(c == 0), stop=(c == CK - 1),
                    )
                if b_idx is None:
                    # ScalarE Copy: the ACT queue is idle pre-stream
                    eng.copy(out_sb, p[:])
                else:
                    eng.tensor_scalar_add(out_sb, p[:], b_all[:, b_idx:b_idx + 1])

            # emission in need order: K q0 + Q q0 gate exp0-h0
            q_proj(1, "xo", 0, kt[:, 0:F], None, "s", 2, nc.scalar)    # K q0
            q_proj(0, "xi", 0, qt[:, 0:F], 0, "s", 2, nc.vector)       # Q q0
            q_proj(1, "xo", 1, kt[:, F:W2], None, "x", 1, nc.scalar)   # K q1
            q_proj(0, "xi", 1, qt[:, F:W2], 0, "s", 2, nc.vector)      # Q q1

            def half_proj(w_idx, xs, hh, out_sb, b_idx, eng):
                """one [128,512] half of an h1 projection in the "x" slot"""
                p = ps.tile([128, F], F32, tag="x", name=f"xp{w_idx}{hh}")
                for c in range(CK):
                    nc.tensor.matmul(
                        p[:], w_all[:, w_idx, c, :], xs[c][1][:, ts(hh, F)],
                        start=(c == 0), stop=(c == CK - 1),
                    )
                dst = out_sb[:, W2 + hh * F: W2 + (hh + 1) * F]
                if b_idx is None:
                    eng.tensor_copy(dst, p[:])
                else:
                    eng.tensor_scalar_add(dst, p[:], b_all[:, b_idx:b_idx + 1])

            # ---- pass 0 ----
            # score(0) uses two separate PSUM half-tiles: PSUM deps are
            # tile-granular, so exp0-h0 starts without waiting the h1 matmul
            pt0 = pts.tile([128, W2], BF16, tag="p0_0", name="p0_0")
            p_tiles[0][0] = pt0
            for hh in range(2):
                sp_h = ps.tile([128, F], F32, tag="s", name=f"s0h{hh}", bufs=2)
                nc.tensor.matmul(
                    sp_h[:], kt[:, 0:128], qt[:, hh * F:(hh + 1) * F],
                    start=True, stop=True,
                )
                nc.scalar.activation(pt0[:, ts(hh, F)], sp_h[:], AFT.Exp,
                                     scale=SCALE)
            for t in range(1, LT):
                if t == LT - 1:
                    # split exp at t=15 shortens the tail
                    score_exp_split(0, t)
                else:
                    score_exp(0, t)
                # AV: starts once v_blk[0] is ready
                if t == 3:
                    for m in range(3):
                        av_mm(0, m)
                elif t >= 4:
                    av_mm(0, t - 1)
                tree(0, t)
                # pass-1 projections ride the PE slack in the "x" slot
                if t == 1:
                    v_group(0, "av")   # av-ring: chains after K h0's first half
                elif t == 3:
                    proj(1, xo_t, 1, kt[:, W2:L], None, "x", nc.vector)  # K h1
                elif t == 5:
                    v_group(1, "x")
                elif t == 9:
                    proj(0, xi_t, 1, qt[:, W2:L], 0, "x", nc.vector)     # Q p1

            # ---- boundary: keep ScalarE fed while pass 0 drains ----
            score_exp(1, 0)
            dn_mm(0, tsum_s[0][1][:], False, False)
            dn_mm(0, p_tiles[0][14][:], False, False)
            score_exp(1, 1)
            av_mm(0, LT - 1, only_h=0)
            dn_mm(0, p_tiles[0][15][:], False, True, only_h=0)
            score_exp(1, 2)
            av_mm(0, LT - 1, only_h=1)
            dn_mm(0, p_tiles[0][15][:], False, True, only_h=1)
            normalize_h(0, 0)
            normalize_h(0, 1)
            tree(1, 2)

            # ---- pass 1 ----
            AV1 = {5: (0,), 6: (1, 2), 7: (3, 4), 8: (5, 6), 9: (7, 8)}
            for t in range(3, LT):
                if t == LT - 1:
                    score_exp_split(1, t)
                else:
                    score_exp(1, t)
                for m in AV1.get(t, (t - 1,) if t >= 10 else ()):
                    av_mm(1, m)
                tree(1, t)
            # tail: only p15's av/dn trail the last exp; muls ride GpSimd
            # so the two reciprocals are back-to-back on DVE
            dn_mm(1, tsum_s[1][1][:], False, False)
            dn_mm(1, p_tiles[1][14][:], False, False)
            av_mm(1, LT - 1, only_h=0)
            dn_mm(1, p_tiles[1][15][:], False, True, only_h=0)
            av_mm(1, LT - 1, only_h=1)
            dn_mm(1, p_tiles[1][15][:], False, True, only_h=1)
            normalize_h(1, 0)
            normalize_h(1, 1)

    nc.compile()
    return nc


def _in_maps(inputs):
    import ml_dtypes

    bf16 = ml_dtypes.bfloat16
    x_inner = np.ascontiguousarray(np.asarray(inputs["x_inner"]).astype(bf16))
    x_outer = np.ascontiguousarray(np.asarray(inputs["x_outer"]).astype(bf16))
    # device layout [p=128, 3, CK, D]: W^T[c, d] split as c = j*128 + p
    w_t = np.stack([
        np.asarray(inputs["Wq"]).astype(np.float32).T,
        np.asarray(inputs["Wk"]).astype(np.float32).T,
        np.asarray(inputs["Wv"]).astype(np.float32).T,
    ])  # [3, C, D]
    w_all = np.ascontiguousarray(
        w_t.reshape(3, C // 128, 128, D).transpose(2, 0, 1, 3)
        .reshape(128, -1).astype(bf16))
    b_all = np.ascontiguousarray(np.stack([
        np.asarray(inputs["bq"], dtype=np.float32),
        np.asarray(inputs["bk"], dtype=np.float32),
        np.asarray(inputs["bv"], dtype=np.float32),
    ], axis=1))
    return [
        {
            "x_inner": x_inner[b],
            "x_outer": x_outer[b],
            "W_all": w_all,
            "b_all": b_all,
        }
        for b in range(B)
    ]


def kernel(**inputs):
    global _COMPILED
    from concourse.bass_utils import run_bass_kernel_spmd

    if _COMPILED is None:
        _COMPILED = _build()
    in_maps = _in_maps(inputs)
    res = run_bass_kernel_spmd(_COMPILED, in_maps, core_ids=list(range(B)))
    # device emits bf16 out^T [D, L]; transpose/upcast on host (pure layout)
    return np.stack(
        [res.results[b]["out"].T.astype(np.float32) for b in range(B)]
    )
